# revision 1
# baseline (speedup 1.0000x reference)
"""Bass/Trainium2 kernel for the GRU seq2seq autoencoder problem.

Sharding: data-parallel over batch (B=128 -> 16 per core, 8 cores), no
cross-core communication.

Per-core layout ("cg-packing"): gate pre-activations live in PSUM as
[128 partitions, 768 free] where partition p = 32*cg + b (cg in 0..3 PE
column group, b in 0..15 local batch; rows 16..31 of each 32-block are
unused) and free f = [r(256) | z(256) | n(256)]; gate dim g (0..1023 per
gate type) maps to (cg = g // 256, idx = g % 256). The 4 col groups are
fed by PE column tiling (tile_position=(0, 32*cg)) so 4 matmuls run
concurrently in the 128x128 array. All elementwise work runs on the full
128 partitions (unused rows compute garbage, which is never read).

Hidden state h (per core [16, 1024]) is kept packed as h_pack[128, 256]
(h[b, cg*256 + idx] = h_pack[32*cg + b, idx]) plus transposed bf16 chunks
hT[128, 2, 128] where hT[i, j, 32*cg + b] = h[b, cg*256 + j*128 + i]; the
slice hT[:, kk%2, 32*(kk//2) : 32*(kk//2)+16] is exactly the stationary
[128, 16] for h-chunk kk (h dims kk*128 .. kk*128+127, natural order).
"""

import sys

sys.path.insert(0, "/opt/trn_rl_repo")

import numpy as np
import ml_dtypes

import concourse.bass as bass
import concourse.mybir as mybir
import concourse.tile as tile
from concourse import bacc
from concourse.masks import make_identity

BF16 = mybir.dt.bfloat16
F32 = mybir.dt.float32
AF = mybir.ActivationFunctionType
OP = mybir.AluOpType

V, E, H = 2048, 512, 1024
B, L = 128, 256
NCORES = 8
BL = B // NCORES          # 16 batch rows per core
CG = 4                    # column groups
G = H // CG               # 256 gate cols per (cg, gate-type)
LD = L - 1                # decoder steps = 255
NT_FC = 32                # fc row-tile count (each covers 8 t slots)
KFC = 20                  # fc contraction chunks (2560 / 128)
P = 128


def row(cg, b):
    return 32 * cg + b


# --------------------------------------------------------------------------
# host-side packing helpers
# --------------------------------------------------------------------------

def pack_w_moving(WT, nk):
    """WT: [K, 3H] = W.T. Returns [nk, CG, 128, 768]; [..., :512] = (r|z)
    columns of this cg, [..., 512:] = n columns."""
    K = WT.shape[0]
    assert K == nk * 128
    out = np.empty((nk, CG, 128, 3 * G), dtype=np.float32)
    for cg in range(CG):
        r = WT[:, 0 * H + cg * G: 0 * H + (cg + 1) * G]
        z = WT[:, 1 * H + cg * G: 1 * H + (cg + 1) * G]
        n = WT[:, 2 * H + cg * G: 2 * H + (cg + 1) * G]
        cat = np.concatenate([r, z, n], axis=1)  # [K, 768]
        out[:, cg] = cat.reshape(nk, 128, 3 * G)
    return out


def pack_bias_rz(b_rz):
    """b_rz: [2H] (r then z). Returns [128, 512]; row 32cg+b = bias row."""
    out = np.zeros((P, 2 * G), dtype=np.float32)
    for cg in range(CG):
        r = b_rz[0 * H + cg * G: 0 * H + (cg + 1) * G]
        z = b_rz[1 * H + cg * G: 1 * H + (cg + 1) * G]
        out[32 * cg: 32 * cg + BL] = np.concatenate([r, z])[None, :]
    return out


def pack_bias_n(b_n):
    """b_n: [H]. Returns [128, 256]."""
    out = np.zeros((P, G), dtype=np.float32)
    for cg in range(CG):
        out[32 * cg: 32 * cg + BL] = b_n[cg * G:(cg + 1) * G][None, :]
    return out


def emb_T_chunks(emb_rows, nsteps):
    """emb_rows: [BL, nsteps, E] -> [128, nsteps, 4, BL] bf16 (partition-major
    x_t^T chunks: [p, t, kc, b] with E-dim = kc*128 + p)."""
    x = emb_rows.transpose(1, 2, 0).reshape(nsteps, 4, 128, BL)
    x = x.transpose(2, 0, 1, 3)              # [128, t, 4, BL]
    return np.ascontiguousarray(x).astype(ml_dtypes.bfloat16)


def prep_shared(W_ih_e, W_hh_e, b_ih_e, b_hh_e, W_ih_d, W_hh_d, b_ih_d,
                b_hh_d, fc_W, fc_b):
    s = {}
    s["w_enc_mov"] = np.concatenate(
        [pack_w_moving(W_ih_e.T, 4), pack_w_moving(W_hh_e.T, 8)], axis=0
    ).astype(ml_dtypes.bfloat16)                       # [12, 4, 128, 768]
    s["w_dec_mov"] = np.concatenate(
        [pack_w_moving(W_ih_d[:, :E].T, 4), pack_w_moving(W_hh_d.T, 8)],
        axis=0,
    ).astype(ml_dtypes.bfloat16)
    s["w_ctx_mov"] = pack_w_moving(W_ih_d[:, E:].T, 8).astype(
        ml_dtypes.bfloat16
    )                                                  # [8, 4, 128, 768]
    s["brz_enc"] = pack_bias_rz((b_ih_e + b_hh_e)[: 2 * H])
    s["bni_enc"] = pack_bias_n(b_ih_e[2 * H:])
    s["bnh_enc"] = pack_bias_n(b_hh_e[2 * H:])
    s["brz_dec"] = pack_bias_rz((b_ih_d + b_hh_d)[: 2 * H])
    s["bni_dec"] = pack_bias_n(b_ih_d[2 * H:])
    s["bnh_dec"] = pack_bias_n(b_hh_d[2 * H:])
    s["fc_w_mov"] = fc_W.T.reshape(KFC, 128, V).astype(ml_dtypes.bfloat16)
    s["fc_b"] = np.ascontiguousarray(fc_b.astype(np.float32))
    return s


def prep_core_inputs(core, seq, emb_enc, emb_dec, shared):
    sl = slice(core * BL, (core + 1) * BL)
    seq_c = np.asarray(seq[sl])                        # [BL, L]
    x_enc = emb_enc[seq_c]                             # [BL, L, E]
    x_enc_T = emb_T_chunks(x_enc, L)
    x_dec_pad = np.zeros((BL, L, E), np.float32)
    x_dec_pad[:, :LD] = emb_dec[seq_c[:, :LD]]
    x_dec_T = emb_T_chunks(x_dec_pad, L)
    m = {"x_enc_T": x_enc_T, "x_dec_T": x_dec_T}
    m.update(shared)
    return m


# --------------------------------------------------------------------------
# device program
# --------------------------------------------------------------------------

def build_program(n_enc=L, n_dec=LD):
    from contextlib import ExitStack

    nc = bacc.Bacc("TRN2", target_bir_lowering=False, debug=False)

    x_enc_d = nc.dram_tensor("x_enc_T", [128, L, 4, BL], BF16, kind="ExternalInput")
    x_dec_d = nc.dram_tensor("x_dec_T", [128, L, 4, BL], BF16, kind="ExternalInput")
    w_enc_d = nc.dram_tensor("w_enc_mov", [12, CG, 128, 3 * G], BF16, kind="ExternalInput")
    w_dec_d = nc.dram_tensor("w_dec_mov", [12, CG, 128, 3 * G], BF16, kind="ExternalInput")
    w_ctx_d = nc.dram_tensor("w_ctx_mov", [8, CG, 128, 3 * G], BF16, kind="ExternalInput")
    brz_enc_d = nc.dram_tensor("brz_enc", [P, 2 * G], F32, kind="ExternalInput")
    bni_enc_d = nc.dram_tensor("bni_enc", [P, G], F32, kind="ExternalInput")
    bnh_enc_d = nc.dram_tensor("bnh_enc", [P, G], F32, kind="ExternalInput")
    brz_dec_d = nc.dram_tensor("brz_dec", [P, 2 * G], F32, kind="ExternalInput")
    bni_dec_d = nc.dram_tensor("bni_dec", [P, G], F32, kind="ExternalInput")
    bnh_dec_d = nc.dram_tensor("bnh_dec", [P, G], F32, kind="ExternalInput")
    fc_w_d = nc.dram_tensor("fc_w_mov", [KFC, 128, V], BF16, kind="ExternalInput")
    fc_b_d = nc.dram_tensor("fc_b", [V], F32, kind="ExternalInput")

    h2T_d = nc.dram_tensor("h2T", [L, 128, 2, P], BF16)   # internal
    ctx_out_d = nc.dram_tensor("ctx_out", [P, G], F32, kind="ExternalOutput")
    out_d = nc.dram_tensor("logits", [NT_FC, 128, V], F32, kind="ExternalOutput")

    with tile.TileContext(nc) as tc:
        with ExitStack() as stack:
            const = stack.enter_context(tc.tile_pool(name="const", bufs=1))
            hpool = stack.enter_context(tc.tile_pool(name="hpool", bufs=2))
            tpool = stack.enter_context(tc.tile_pool(name="tpool", bufs=2))

            ident = const.tile([P, P], BF16)
            make_identity(nc, ident)

            state = {}

            def step(t, x_fn, w_sb, biases, ps, dma_h2_t=None):
                brz_b, bni_b, bnh_b = biases
                ps_rz, ps_ngi, ps_ngh, ps_tr = ps
                hT_prev = state["hT"]
                h_prev = state["h_f32"]

                p_rz = ps_rz.tile([P, 2 * G], F32, tag="p_rz")
                p_ngi = ps_ngi.tile([P, G], F32, tag="p_ngi")
                p_ngh = ps_ngh.tile([P, G], F32, tag="p_ngh")

                for kk in range(12):
                    if kk < 4:
                        stat = x_fn(t, kk)
                    else:
                        kh = kk - 4
                        stat = hT_prev[:, kh % 2, 32 * (kh // 2): 32 * (kh // 2) + BL]
                    for acg in range(CG):
                        psl = slice(32 * acg, 32 * acg + BL)
                        nc.tensor.matmul(
                            p_rz[psl, :], stat, w_sb[:, kk, acg, 0:2 * G],
                            start=(kk == 0), stop=(kk == 11),
                            tile_position=(0, 32 * acg),
                        )
                        if kk < 4:
                            nc.tensor.matmul(
                                p_ngi[psl, :], stat, w_sb[:, kk, acg, 2 * G:],
                                start=(kk == 0), stop=(kk == 3),
                                tile_position=(0, 32 * acg),
                            )
                        else:
                            nc.tensor.matmul(
                                p_ngh[psl, :], stat, w_sb[:, kk, acg, 2 * G:],
                                start=(kk == 4), stop=(kk == 11),
                                tile_position=(0, 32 * acg),
                            )

                # gates (full 128 partitions; unused rows are garbage)
                rz_pre = tpool.tile([P, 2 * G], F32, tag="rz_pre")
                nc.vector.tensor_add(rz_pre, p_rz, brz_b)
                rz_s = tpool.tile([P, 2 * G], F32, tag="rz_s")
                nc.scalar.activation(rz_s, rz_pre, AF.Sigmoid)

                t1 = tpool.tile([P, G], F32, tag="t1")
                nc.vector.tensor_add(t1, p_ngh, bnh_b)        # h_n + b_hh_n
                t2 = tpool.tile([P, G], F32, tag="t2")
                nc.vector.tensor_mul(t2, t1, rz_s[:, 0:G])    # r * (.)
                e1 = tpool.tile([P, G], F32, tag="e1")
                nc.vector.tensor_add(e1, p_ngi, bni_b)        # i_n + b_ih_n
                t3 = tpool.tile([P, G], F32, tag="t3")
                nc.vector.tensor_add(t3, t2, e1)
                n_t = tpool.tile([P, G], F32, tag="n_t")
                nc.scalar.activation(n_t, t3, AF.Tanh)

                u1 = tpool.tile([P, G], F32, tag="u1")
                nc.gpsimd.tensor_tensor(u1, h_prev, n_t, OP.subtract)
                u2 = tpool.tile([P, G], F32, tag="u2")
                nc.gpsimd.tensor_tensor(u2, u1, rz_s[:, G:2 * G], OP.mult)
                h_new = hpool.tile([P, G], F32, tag="hf32")
                nc.gpsimd.tensor_tensor(h_new, u2, n_t, OP.add)

                h_bf = tpool.tile([P, G], BF16, tag="h_bf")
                nc.scalar.copy(h_bf, h_new)

                p_t = ps_tr.tile([P, 2, P], BF16, tag="p_t")
                nc.tensor.transpose(p_t[:, 0, :], h_bf[:, 0:128], ident)
                nc.tensor.transpose(p_t[:, 1, :], h_bf[:, 128:256], ident)
                hT_new = hpool.tile([P, 2, P], BF16, tag="hT")
                nc.vector.tensor_copy(hT_new[:, 0, :], p_t[:, 0, :])
                nc.vector.tensor_copy(hT_new[:, 1, :], p_t[:, 1, :])

                if dma_h2_t is not None:
                    nc.gpsimd.dma_start(out=h2T_d[dma_h2_t], in_=hT_new)

                state["h_f32"] = h_new
                state["hT"] = hT_new

            # ================= recurrence phases =================
            with ExitStack() as rstack:
                wpool = rstack.enter_context(tc.tile_pool(name="wpool", bufs=1))
                xpool = rstack.enter_context(tc.tile_pool(name="xpool", bufs=1))
                ps_rz = rstack.enter_context(tc.tile_pool(name="ps_rz", bufs=2, space="PSUM"))
                ps_ngi = rstack.enter_context(tc.tile_pool(name="ps_ngi", bufs=2, space="PSUM"))
                ps_ngh = rstack.enter_context(tc.tile_pool(name="ps_ngh", bufs=2, space="PSUM"))
                ps_tr = rstack.enter_context(tc.tile_pool(name="ps_tr", bufs=2, space="PSUM"))
                ps = (ps_rz, ps_ngi, ps_ngh, ps_tr)

                # pre-zero the psum slots so unused rows stay finite
                for _ in range(2):
                    z1 = ps_rz.tile([P, 2 * G], F32, tag="p_rz")
                    nc.vector.memset(z1, 0.0)
                    z2 = ps_ngi.tile([P, G], F32, tag="p_ngi")
                    nc.vector.memset(z2, 0.0)
                    z3 = ps_ngh.tile([P, G], F32, tag="p_ngh")
                    nc.vector.memset(z3, 0.0)

                # ---------------- encoder ----------------
                x_sb = xpool.tile([128, L, 4, BL], BF16, tag="xsb")
                nc.gpsimd.dma_start(out=x_sb, in_=x_enc_d[:, :, :, :])
                we_sb = wpool.tile([128, 12, CG, 3 * G], BF16, tag="wsb")
                nc.gpsimd.dma_start(out=we_sb, in_=w_enc_d.rearrange("k c p f -> p k c f"))

                brz_enc = const.tile([P, 2 * G], F32)
                bni_enc = const.tile([P, G], F32)
                bnh_enc = const.tile([P, G], F32)
                nc.gpsimd.dma_start(out=brz_enc, in_=brz_enc_d[:, :])
                nc.gpsimd.dma_start(out=bni_enc, in_=bni_enc_d[:, :])
                nc.gpsimd.dma_start(out=bnh_enc, in_=bnh_enc_d[:, :])

                h0 = hpool.tile([P, G], F32, tag="hf32")
                nc.vector.memset(h0, 0.0)
                hT0 = hpool.tile([P, 2, P], BF16, tag="hT")
                nc.vector.memset(hT0, 0.0)
                state["h_f32"] = h0
                state["hT"] = hT0

                enc_biases = (brz_enc, bni_enc, bnh_enc)
                for t in range(n_enc):
                    step(t, lambda t, kk: x_sb[:, t, kk, :], we_sb, enc_biases, ps)

                ctx_f32 = state["h_f32"]
                ctx_T = state["hT"]
                nc.gpsimd.dma_start(out=ctx_out_d[:, :], in_=ctx_f32)

                # ---------- ctx projection + decoder bias prep ----------
                wctx_sb = wpool.tile([128, 12, CG, 3 * G], BF16, tag="wsb")
                nc.gpsimd.dma_start(
                    out=wctx_sb[:, 0:8], in_=w_ctx_d.rearrange("k c p f -> p k c f")
                )
                pc_rz = ps_rz.tile([P, 2 * G], F32, tag="p_rz")
                pc_n = ps_ngi.tile([P, G], F32, tag="p_ngi")
                for kk in range(8):
                    stat = ctx_T[:, kk % 2, 32 * (kk // 2): 32 * (kk // 2) + BL]
                    for acg in range(CG):
                        psl = slice(32 * acg, 32 * acg + BL)
                        nc.tensor.matmul(
                            pc_rz[psl, :], stat, wctx_sb[:, kk, acg, 0:2 * G],
                            start=(kk == 0), stop=(kk == 7),
                            tile_position=(0, 32 * acg),
                        )
                        nc.tensor.matmul(
                            pc_n[psl, :], stat, wctx_sb[:, kk, acg, 2 * G:],
                            start=(kk == 0), stop=(kk == 7),
                            tile_position=(0, 32 * acg),
                        )
                brz_dec_b = const.tile([P, 2 * G], F32)
                bni_dec_b = const.tile([P, G], F32)
                bnh_dec = const.tile([P, G], F32)
                nc.gpsimd.dma_start(out=brz_dec_b, in_=brz_dec_d[:, :])
                nc.gpsimd.dma_start(out=bni_dec_b, in_=bni_dec_d[:, :])
                nc.gpsimd.dma_start(out=bnh_dec, in_=bnh_dec_d[:, :])
                brz_dec = const.tile([P, 2 * G], F32)
                nc.vector.tensor_add(brz_dec, pc_rz, brz_dec_b)
                bni_dec = const.tile([P, G], F32)
                nc.vector.tensor_add(bni_dec, pc_n, bni_dec_b)

                # fc ctx stationary tiles: ctx^T chunk k replicated over 8 t
                ctx_fc = const.tile([128, 8, 8, BL], BF16)
                for k in range(8):
                    for tt in range(8):
                        nc.vector.tensor_copy(
                            ctx_fc[:, k, tt, :],
                            ctx_T[:, k % 2, 32 * (k // 2): 32 * (k // 2) + BL],
                        )

                # ---------------- decoder ----------------
                xd_sb = xpool.tile([128, L, 4, BL], BF16, tag="xsb")
                nc.gpsimd.dma_start(out=xd_sb, in_=x_dec_d[:, :, :, :])
                wd_sb = wpool.tile([128, 12, CG, 3 * G], BF16, tag="wsb")
                nc.gpsimd.dma_start(out=wd_sb, in_=w_dec_d.rearrange("k c p f -> p k c f"))

                zpad = const.tile([P, 2, P], BF16)
                nc.vector.memset(zpad, 0.0)
                nc.gpsimd.dma_start(out=h2T_d[L - 1], in_=zpad)

                dec_biases = (brz_dec, bni_dec, bnh_dec)
                for t in range(n_dec):
                    step(t, lambda t, kk: xd_sb[:, t, kk, :], wd_sb, dec_biases,
                         ps, dma_h2_t=t)

            # ---------------- fc projection ----------------
            with tc.tile_pool(name="fcw", bufs=1) as fcw_pool, \
                 tc.tile_pool(name="fcpool", bufs=3) as fcpool, \
                 tc.tile_pool(name="ps_fc", bufs=2, space="PSUM") as ps_fc:
                fcw_sb = fcw_pool.tile([128, KFC, V], BF16)
                nc.gpsimd.dma_start(out=fcw_sb, in_=fc_w_d.rearrange("k p v -> p k v"))
                fcb_sb = fcw_pool.tile([128, V], F32)
                fcb_ap = fc_b_d[:]
                fcb_bcast = bass.AP(
                    tensor=fcb_ap.tensor, offset=fcb_ap.offset,
                    ap=[[0, 128], [1, V]],
                )
                nc.gpsimd.dma_start(out=fcb_sb, in_=fcb_bcast)

                for m in range(NT_FC):
                    stat = fcpool.tile([128, 12, 8, BL], BF16, tag="fcstat")
                    for kc in range(4):
                        nc.gpsimd.dma_start(
                            out=stat[:, kc],
                            in_=x_dec_d[:, m * 8:(m + 1) * 8, kc, :],
                        )
                    for kh in range(8):
                        nc.gpsimd.dma_start(
                            out=stat[:, 4 + kh],
                            in_=h2T_d[m * 8:(m + 1) * 8, :, kh % 2,
                                      32 * (kh // 2): 32 * (kh // 2) + BL]
                            .rearrange("t p b -> p t b"),
                        )
                    p_fc = ps_fc.tile([128, V], F32, tag="p_fc")
                    for k in range(KFC):
                        if k < 12:
                            st = stat[:, k, :, :].rearrange("p t b -> p (t b)")
                        else:
                            st = ctx_fc[:, k - 12, :, :].rearrange("p t b -> p (t b)")
                        for nt in range(4):
                            nc.tensor.matmul(
                                p_fc[:, nt * 512:(nt + 1) * 512], st,
                                fcw_sb[:, k, nt * 512:(nt + 1) * 512],
                                start=(k == 0), stop=(k == KFC - 1),
                            )
                    o_t = fcpool.tile([128, V], F32, tag="o_t")
                    for nt in range(4):
                        sl = slice(nt * 512, (nt + 1) * 512)
                        nc.vector.tensor_add(o_t[:, sl], p_fc[:, sl], fcb_sb[:, sl])
                    nc.gpsimd.dma_start(out=out_d[m], in_=o_t)

    nc.finalize()
    return nc


# --------------------------------------------------------------------------
# full pipeline
# --------------------------------------------------------------------------

def make_in_maps(inputs):
    seq = np.asarray(inputs["seq"])
    shared = prep_shared(
        np.asarray(inputs["W_ih_e"], np.float32), np.asarray(inputs["W_hh_e"], np.float32),
        np.asarray(inputs["b_ih_e"], np.float32), np.asarray(inputs["b_hh_e"], np.float32),
        np.asarray(inputs["W_ih_d"], np.float32), np.asarray(inputs["W_hh_d"], np.float32),
        np.asarray(inputs["b_ih_d"], np.float32), np.asarray(inputs["b_hh_d"], np.float32),
        np.asarray(inputs["fc_W"], np.float32), np.asarray(inputs["fc_b"], np.float32),
    )
    emb_enc = np.asarray(inputs["emb_enc"], np.float32)
    emb_dec = np.asarray(inputs["emb_dec"], np.float32)
    return [prep_core_inputs(c, seq, emb_enc, emb_dec, shared)
            for c in range(NCORES)]


def assemble(results):
    outputs = np.zeros((B, L, V), np.float32)
    context = np.zeros((B, H), np.float32)
    for c in range(NCORES):
        r = results[c]
        co = r["ctx_out"].reshape(CG, 32, G)[:, :BL]           # [4, 16, 256]
        context[c * BL:(c + 1) * BL] = co.transpose(1, 0, 2).reshape(BL, H)
        lg = r["logits"].reshape(NT_FC * 8, BL, V).transpose(1, 0, 2)
        outputs[c * BL:(c + 1) * BL, 1:, :] = lg[:, :LD, :]
    mu = np.zeros((B, H), np.float32)
    log_var = np.zeros((B, H), np.float32)
    return outputs, context, mu, log_var


def run(inputs, nc=None, n_enc=L, n_dec=LD):
    from concourse.bass_utils import run_bass_kernel_spmd

    in_maps = make_in_maps(inputs)
    if nc is None:
        nc = build_program(n_enc, n_dec)
    res = run_bass_kernel_spmd(nc, in_maps, list(range(NCORES)))
    return assemble(res.results)


# --------------------------------------------------------------------------
# harness entry point
# --------------------------------------------------------------------------

_NC_CACHE = {}


def kernel(**inputs):
    """Full-input entry: shards across 8 NeuronCores internally, returns the
    full outputs (outputs, context, mu, log_var) matching reference()."""
    if "nc" not in _NC_CACHE:
        _NC_CACHE["nc"] = build_program()
    return run(inputs, nc=_NC_CACHE["nc"])
